# revision 55
# baseline (speedup 1.0000x reference)
"""AttentiveMMDPrompt.compute_attn_weight kernel for 8 Trainium2 NeuronCores.

Strategy (data-parallel over episodes b=8, one episode per core):

  Per episode the heavy compute is the local projection l = local_f @ Wk^T
  (14700x640 @ 640x640).  Everything downstream needs only per-token
  scalars: |l|^2, the five raw scores l.ghat_g, and l.mean — all columns of
  one widened matmul Z = x @ R.

  The widened matmul runs in fp8 (e4m3) with DoubleRow perf mode: 0.5
  cycles/row, 2x the bf16/f32r rate.  fp8 alone is accurate enough for
  |l|^2 (quantization errors average out over 640 terms) but NOT for the
  score columns, so scores use a 3-term fp8 split that reaches bf16-level
  accuracy at fp8 speed:

    x  = a + b/32           a = fp8(x),         b = fp8(32(x - a))
    rs = rs8/32 + r2/1024   rs8 = fp8(32 rs),   r2 = fp8(1024(rs - rs8/32))
    x@rs ~= [a@rs8 + (a@r2 + b@rs8)/32] / 32    (second-order terms dropped)

  PSUM Z layout per 128-token chunk (two banks; a 652-col single matmul
  would cross the 2KB bank boundary, which the ISA rejects):
    bank0  0:320    kappa*l (norm, lo)  - squared+accumulated on ACT
    bank1  512:832  kappa*l (norm, hi)  - squared via DVE bn_stats, with
                                          the mean/var fold-in batched at
                                          phase level
    bank1  832:844  A1 | A2+B           - A1/A2 ride the A matmuls free;
                                          5 single-subtile fp8 B matmuls
                                          accumulate into the A2 columns
                                          (same 1/32 downstream scale;
                                          singles beat DoubleRow pairs here
                                          because their tiny streams cannot
                                          hide weight loads, and single
                                          loads are half the size)
  DVE copies the 12 score cols out per chunk; ln/exp on ACT replace
  sqrt+reciprocal so Square/Ln/Exp/Copy share one ACT function table
  (each table swap stalls ACT ~1.3us).

  Softmax groups (196 tokens) complete a few chunks after their last
  token chunk, so the whole phase epilogue (softmax stats, exp), the
  group sums, 1/sum and the broadcast-normalize matmuls for phase p are
  emitted as small steps interleaved between the chunks of phase p+1 —
  no engine sees a burst and the serial tail shrinks from ~75 chunks of
  work to ~16.  Group-sum matmuls are flipped (stationary = ebuf chunk,
  moving = indicator chunk, both bf16) so the big weight load sits on
  the cheap side; broadcast-normalize multiplies run batched 8 chunks
  per DVE op (PSUM-access setup dominates small ops).

  Device inputs per core (everything pre-laid-out on the host):
    xa     [128, 115, 6, 128] fp8 - a = fp8(x), partition-major blocks,
                                    c zero-padded 640->768 so each
                                    5-chunk group DMA is one contiguous
                                    3.84KB run per partition
    xb     [128, 115, 6, 128] fp8 - b = fp8(32(x - a)), same blocking
    rmat   [6, 128, 652] fp8     - [8*Wk^T | rs8 | r2], subtile 5 zeros
    consts [6] f32               - [32*ghat_g.mean (5) | 1024*a^2(|mean|^2+eps)]
    bmat   [128, 115, 75] bf16   - group indicator chunks, partition-major
    btmat  [75, 14720] bf16      - its transpose
  Output per core:
    O [5, 14720] f32 - normalized attention, token-major per g; host slices
    the 20 pad tokens and reshapes to [75, 5, 196, 1].
"""

import numpy as np
import ml_dtypes

import bass_rust
import concourse.bass as bass
import concourse.mybir as mybir
import concourse.tile as tile
from concourse.bass_utils import run_bass_kernel_spmd
from concourse.masks import make_identity

# Problem shapes (hardcoded per contract).
B, NG, NL, NF, C = 8, 5, 75, 196, 640
ALPHA, EPS = 0.1, 1e-12
NT = NL * NF            # 14700 tokens per episode
TCH = 115               # token chunks of 128
NTP = TCH * 128         # 14720 (padded)
CCH = 5                 # contraction chunks of 128 (C = 640)
KAPPA = 8.0             # fp8 prescale of Wk^T (norm path)
ZN = 640                # norm columns of Z
ZP = 652                # z columns: 640 norm | 6 A1 | 6 A2 (+6 B separate)
ZA = 320                # bank0 norm cols; bank1 holds 320:652 (+6 B cols)
G = 5                   # token chunks per DMA group (115 = 23*5)
NSLOT = 3               # x-stream SBUF slots
DEFER = 3               # chunks into phase p+1 before phase-p reductions
STEPS = 4               # deferred-work steps interleaved per chunk
PHASE_ENDS = [25, 50, 75, 100, TCH]
F32 = mybir.dt.float32
BF16 = mybir.dt.bfloat16
FP8 = mybir.dt.float8e4
AF = mybir.ActivationFunctionType
ALU = mybir.AluOpType
DR = mybir.MatmulPerfMode.DoubleRow
E4M3 = ml_dtypes.float8_e4m3


def _group_bounds():
    """Per phase: groups fully summed and chunks fully normalizable."""
    L, T = [], []
    for E in PHASE_ENDS:
        l = (128 * E) // NF          # groups with last token < 128E
        L.append(min(l, NL))
        if E == TCH:
            T.append(TCH)            # pad tokens have all-zero indicators
        else:
            T.append((NF * min(l, NL) - 128) // 128 + 1)
    return L, T


def _split_multi_waits(nc: bass.Bass) -> None:
    """Rewrite the BIR so no instruction carries more than one sem wait.

    The walrus build in this container rejects instructions with more than
    one sync-wait command (CoreV3 setupSyncWait, all encodings).  Extra
    waits are hoisted onto no-op instructions inserted immediately before
    the owner on the same engine: waits execute in program order per
    engine sequencer, so satisfying them one instruction earlier on the
    same engine is semantically identical.
    """
    for f in nc.m.functions:
        for b in f.blocks:
            insts = list(b.instructions)
            out = []
            changed = False
            for inst in insts:
                si = inst.sync_info
                if si is not None and len(si.on_wait) > 1:
                    waits = list(si.on_wait)
                    for w in waits[:-1]:
                        nop = mybir.InstNoOp(
                            name=nc.get_next_instruction_name(), ins=[], outs=[]
                        )
                        nop.engine = inst.engine
                        nop.sync_info = bass_rust.SyncInfo(
                            on_wait=[w], on_update=[]
                        )
                        nc.register_instruction(nop)
                        out.append(nop)
                    inst.sync_info = bass_rust.SyncInfo(
                        on_wait=[waits[-1]], on_update=list(si.on_update)
                    )
                    changed = True
                out.append(inst)
            if changed:
                b.instructions = out


def _build_program() -> bass.Bass:
    nc = bass.Bass(
        "TRN2",
        target_bir_lowering=False,
        debug=False,
        enable_asserts=True,
        num_devices=B,
    )
    xa = nc.dram_tensor("xa", [128, TCH, 6, 128], FP8, kind="ExternalInput")
    xb = nc.dram_tensor("xb", [128, TCH, 6, 128], FP8, kind="ExternalInput")
    rmat = nc.dram_tensor("rmat", [6, 128, ZP], FP8, kind="ExternalInput")
    consts = nc.dram_tensor("consts", [NG + 1], F32, kind="ExternalInput")
    bmat = nc.dram_tensor("bmat", [128, TCH, NL], BF16, kind="ExternalInput")
    btmat = nc.dram_tensor("btmat", [NL, NTP], BF16, kind="ExternalInput")
    O = nc.dram_tensor("O", [NG, NTP], F32, kind="ExternalOutput")

    LB, TB = _group_bounds()

    with tile.TileContext(nc, num_cores=B) as tc:
        with (
            tc.tile_pool(name="singles", bufs=1) as singles,
            tc.tile_pool(name="zpsum", bufs=3, space="PSUM") as zpsum,
            tc.tile_pool(name="gpsum", bufs=1, space="PSUM") as gpsum,
            tc.tile_pool(name="tpsum", bufs=1, space="PSUM") as tpsum,
        ):
            # ---- one-time loads -------------------------------------------------
            # x streams: persistent slot buffers.  Subtile 5 (the DoubleRow
            # partner of contraction chunk 4) is zero-padded on the host so
            # each 5-chunk group DMA is fully contiguous on both sides.
            xas = singles.tile([128, NSLOT, G, 6, 128], FP8)
            xbs = singles.tile([128, NSLOT, G, 6, 128], FP8)
            rm = singles.tile([128, 6, ZP], FP8)
            rmr = rmat.rearrange("s p z -> p s z")

            # Startup order matters doubly: each dma_start costs ~0.6us of
            # SERIAL descriptor-issue time on the Sync engine, so keep the
            # issue count low AND put everything chunk 0 needs (its x piece
            # and ALL of rm) ahead of bulkier transfers.
            nc.sync.dma_start(out=rm[:, 0:2, :], in_=rmr[:, 0:2, :])
            for src, dst in ((xa, xas), (xb, xbs)):
                nc.sync.dma_start(out=dst[:, 0, 0, :, :], in_=src[:, 0, :, :])
            nc.sync.dma_start(out=rm[:, 2:6, :], in_=rmr[:, 2:6, :])
            for src, dst in ((xa, xas), (xb, xbs)):
                nc.sync.dma_start(
                    out=dst[:, 0, 1:G, :, :], in_=src[:, 1:G, :, :]
                )

            bsb = singles.tile([128, TCH, NL], BF16)
            btsb = singles.tile([NL, TCH, 128], BF16)

            cg = singles.tile([128, NG], F32)
            nc.sync.dma_start(out=cg, in_=consts[0:NG].partition_broadcast(128))
            m2e = singles.tile([128, 1], F32)
            nc.sync.dma_start(
                out=m2e, in_=consts[NG : NG + 1].partition_broadcast(128)
            )

            ident = singles.tile([128, 128], F32)
            make_identity(nc, ident)

            # ---- persistent per-token stats -------------------------------------
            ssa = singles.tile([128, TCH], F32)       # kappa^2 |l|^2 per token
            sq12 = singles.tile([128, 12, TCH], F32)  # raw A1 | A2+B score cols
            sclm = singles.tile([128, 6, TCH], F32)   # combined 32*(x@rs)
            sqa = singles.tile([128, ZA], F32)        # ACT square scratch
            bno = singles.tile([128, 2, 3, TCH], F32)  # DVE bn_stats out
            bnm = singles.tile([128, 2, TCH], F32)    # bn mini scratch
            tmp0 = singles.tile([128, TCH], F32)
            nrm = singles.tile([128, TCH], F32)
            inv = singles.tile([128, TCH], F32)
            sfin = singles.tile([128, NG, TCH], F32)
            ebuf = singles.tile([128, NG, TCH], BF16)
            abuf = singles.tile([128, NG, TCH], BF16)
            obuf = singles.tile([TCH, NG, 128], F32)
            gsum = singles.tile([NG, NL], F32)        # group sums (g-major)
            rgsT = singles.tile([NG, NL], F32)        # 1/gsum (g-major)
            rgs = singles.tile([NL, NG], BF16)        # 1/gsum, group-major

            nc.vector.memset(gsum, 0)
            nc.vector.memset(rgsT, 0)

            def emit_chunk(t):
                gi, j = divmod(t, G)
                slot = gi % NSLOT
                if j == 0 and gi > 0:
                    for src, dst in ((xa, xas), (xb, xbs)):
                        nc.sync.dma_start(
                            out=dst[:, slot, :, :, :],
                            in_=src[:, G * gi : G * (gi + 1), :, :],
                        )
                pz = zpsum.tile([128, 1024], F32, tag="pz")
                for k in range(3):
                    pr = slice(2 * k, 2 * k + 2)
                    nc.tensor.matmul(
                        pz[:, 0:ZA],
                        xas[:, slot, j, pr, :],
                        rm[:, pr, 0:ZA],
                        start=(k == 0),
                        stop=(k == 2),
                        perf_mode=DR,
                    )
                    nc.tensor.matmul(
                        pz[:, 512 : 512 + (ZP - ZA)],
                        xas[:, slot, j, pr, :],
                        rm[:, pr, ZA:ZP],
                        start=(k == 0),
                        stop=(k == 2),
                        perf_mode=DR,
                    )
                # b@rs8 accumulates straight into the A2 columns (the two
                # correction terms share the same downstream 1/32 scale):
                # bank1's start-matmul zeroed 838:844, so these just add.
                for k in range(CCH):
                    nc.tensor.matmul(
                        pz[:, 838:844],
                        xbs[:, slot, j, k, :],
                        rm[:, k, ZN : ZN + 6],
                        start=False,
                        stop=(k == CCH - 1),
                        skip_group_check=True,
                    )
                # |kappa l|^2: ACT squares bank0 (320 cols, accum), DVE
                # covers bank1 (320 cols) via bn_stats; the mean/var fold-in
                # happens once per phase.
                nc.scalar.activation(
                    sqa,
                    pz[:, 0:ZA],
                    AF.Square,
                    accum_out=ssa[:, t : t + 1],
                )
                nc.vector.bn_stats(bno[:, :, :, t], pz[:, 512 : 512 + ZA])
                # raw score columns A1 | A2+B (12 cols, one copy).
                nc.vector.tensor_copy(sq12[:, :, t], pz[:, 832:844])

            def deferred_work(p):
                """Epilogue + reduction work for phase p, yielded as small
                steps to interleave between phase p+1's chunks so no engine
                sees a burst: softmax stats, exp, group-sum matmuls, 1/sum
                for newly-final groups, then broadcast + normalize for
                chunks whose groups are all final."""
                S = PHASE_ENDS[p - 1] if p else 0
                E = PHASE_ENDS[p]
                sl = slice(S, E)
                # fold the DVE bn halves into ssa: ssa += sum_h cv_h + 160 mu_h^2
                nc.vector.tensor_mul(
                    bnm[:, :, sl], bno[:, :, 1, sl], bno[:, :, 1, sl]
                )
                yield
                nc.vector.scalar_tensor_tensor(
                    out=bnm[:, :, sl],
                    in0=bnm[:, :, sl],
                    scalar=float(ZA // 2),
                    in1=bno[:, :, 2, sl],
                    op0=ALU.mult,
                    op1=ALU.add,
                )
                yield
                nc.vector.tensor_add(bnm[:, 0, sl], bnm[:, 0, sl], bnm[:, 1, sl])
                yield
                nc.vector.tensor_add(ssa[:, sl], ssa[:, sl], bnm[:, 0, sl])
                yield
                # combined score: sclm = A1 + (A2 + B)/32  (= 32 * x@rs)
                nc.vector.scalar_tensor_tensor(
                    out=sclm[:, :, sl],
                    in0=sq12[:, 6:12, sl],
                    scalar=1.0 / 32.0,
                    in1=sq12[:, 0:6, sl],
                    op0=ALU.mult,
                    op1=ALU.add,
                )
                yield
                nc.vector.scalar_tensor_tensor(
                    out=nrm[:, sl],
                    in0=sclm[:, 5, sl],
                    scalar=-1.0 / 16.0,
                    in1=ssa[:, sl],
                    op0=ALU.mult,
                    op1=ALU.add,
                )
                yield
                # 1/(32 a |l-mean|) = exp(-ln(scale*nrm + bias)/2): ln and
                # exp share one ACT function table with square/copy, so the
                # engine never swaps tables (a 1.3us stall each time).
                nc.scalar.activation(
                    tmp0[:, sl],
                    nrm[:, sl],
                    AF.Ln,
                    bias=m2e[:, 0:1],
                    scale=float(1024.0 * ALPHA * ALPHA / (KAPPA * KAPPA)),
                )
                yield
                nc.scalar.activation(inv[:, sl], tmp0[:, sl], AF.Exp, scale=-0.5)
                yield
                for g in range(NG):
                    nc.vector.scalar_tensor_tensor(
                        out=sfin[:, g, sl],
                        in0=sclm[:, g, sl],
                        scalar=cg[:, g : g + 1],
                        in1=inv[:, sl],
                        op0=ALU.subtract,
                        op1=ALU.mult,
                    )
                    yield
                nc.scalar.activation(ebuf[:, :, sl], sfin[:, :, sl], AF.Exp)
                yield
                # group sums for this phase's chunks
                gst = gpsum.tile([NG, NL], F32, tag="gs")
                for t in range(S, E):
                    nc.tensor.matmul(
                        gst[:, :],
                        ebuf[:, :, t],
                        bsb[:, t, :],
                        start=(t == S),
                        stop=(t == E - 1),
                    )
                    if t > S:
                        yield
                # fold into running sums; reciprocal of newly-final rows
                nc.vector.tensor_add(gsum, gsum, gst[:, :])
                l0 = LB[p - 1] if p else 0
                l1 = LB[p]
                nc.vector.reciprocal(rgsT[:, l0:l1], gsum[:, l0:l1])
                yield
                tpr = tpsum.tile([NL, NG], F32, tag="tail")
                nc.tensor.transpose(tpr[:, :], rgsT[:, :], ident[:NG, :NG])
                nc.vector.tensor_copy(rgs, tpr[:, :])
                yield
                # broadcast 1/sum to tokens + normalize, batched 8 chunks
                # per DVE multiply (PSUM-access setup dominates small ops)
                t0 = TB[p - 1] if p else 0
                ts = list(range(t0, TB[p]))
                for i in range(0, len(ts), 8):
                    bt = ts[i : i + 8]
                    r2 = tpsum.tile([128, 8, NG], F32, tag="tail")
                    for k2, t in enumerate(bt):
                        nc.tensor.matmul(
                            r2[:, k2, :], btsb[:, t, :], rgs[:, :],
                            start=True, stop=True,
                        )
                        yield
                    nc.vector.tensor_mul(
                        abuf[:, :, bt[0] : bt[0] + len(bt)],
                        ebuf[:, :, bt[0] : bt[0] + len(bt)],
                        r2[:, 0 : len(bt), :].rearrange("p t g -> p g t"),
                    )
                    yield

            # ---- main pass ------------------------------------------------------
            pendings = []
            ph_start = 0
            for p, ph_end in enumerate(PHASE_ENDS):
                for t in range(ph_start, ph_end):
                    emit_chunk(t)
                    # Drain deferred reduction work, a few steps per chunk.
                    # The queue persists across phase boundaries so a short
                    # phase never forces a serial burst of leftover steps.
                    if t >= ph_start + DEFER:
                        budget = STEPS
                        while pendings and budget > 0:
                            if next(pendings[0], StopIteration) is StopIteration:
                                pendings.pop(0)
                            else:
                                budget -= 1
                    if t == ph_start + 1:
                        # indicator slices for THIS phase's deferred work,
                        # issued piecewise so nothing waits on one big blob
                        # and startup x-groups keep queue priority.
                        nc.sync.dma_start(
                            out=bsb[:, ph_start:ph_end, :],
                            in_=bmat[:, ph_start:ph_end, :],
                        )
                        bt0 = TB[p - 1] if p else 0
                        nc.sync.dma_start(
                            out=btsb[:, bt0 : TB[p], :],
                            in_=btmat.rearrange("l (t p) -> l t p", p=128)[
                                :, bt0 : TB[p], :
                            ],
                        )
                pendings.append(deferred_work(p))
                ph_start = ph_end
            for gen in pendings:
                for _ in gen:
                    pass

            # ---- transpose to token-major and store -----------------------------
            # bf16 transposes run at 1 cyc/row, half the f32 rate.
            identb = singles.tile([128, 128], BF16)
            nc.vector.tensor_copy(identb, ident)
            for g in range(NG):
                tp = tpsum.tile([TCH, 128], BF16, tag="tail")
                nc.tensor.transpose(tp[:, :], abuf[:, g, :], identb[:, :])
                nc.scalar.copy(obuf[:, g, :], tp[:, :])
            nc.sync.dma_start(
                out=O.rearrange("g (t p) -> t g p", p=128), in_=obuf
            )

    _split_multi_waits(nc)
    return nc


_PROGRAM_CACHE: list = []
LAST_RESULTS: list = []


def _block_x(t8: np.ndarray) -> np.ndarray:
    """[NT, C] fp8 -> [128p, TCH, 6s, 128i] partition-major blocks.

    The c dim is zero-padded 640 -> 768 so subtile 5 (the DoubleRow
    partner of contraction chunk 4) streams as real zeros and each
    5-chunk group is one fully contiguous 3840B run per partition."""
    buf = np.zeros((NTP, 768), dtype=E4M3)
    buf[:NT, :C] = t8
    return np.ascontiguousarray(
        buf.reshape(TCH, 128, 6, 128).transpose(3, 0, 2, 1)
    )


def _host_prep(global_f, local_f, Wq, Wk):
    """Per-episode host-side constant prep + layout marshaling -> in_maps."""
    gf = np.asarray(global_f, dtype=np.float32)
    lf = np.asarray(local_f, dtype=np.float32)
    Wq64 = np.asarray(Wq, dtype=np.float64)
    Wk64 = np.asarray(Wk, dtype=np.float64)

    # Episode-independent device tensors.
    tok = np.arange(NTP)
    grp = tok // NF
    bmat_full = ((grp[:, None] == np.arange(NL)[None, :]) & (tok[:, None] < NT))
    bmat_full = bmat_full.astype(ml_dtypes.bfloat16)        # [14720, 75]
    bmat = np.ascontiguousarray(
        bmat_full.reshape(TCH, 128, NL).transpose(1, 0, 2)
    )                                                       # [128, 115, 75]
    btmat = np.ascontiguousarray(bmat_full.T)               # [75, 14720]

    in_maps = []
    for bi in range(B):
        x = lf[bi].reshape(NT, C)
        a8 = x.astype(E4M3)
        b8 = (32.0 * (x - a8.astype(np.float32))).astype(E4M3)

        x64 = x.astype(np.float64)
        q = gf[bi].astype(np.float64) @ Wq64.T              # [5, 640]
        mean = (q.sum(0) + x64.sum(0) @ Wk64.T) / (NG + NT)
        gc_ = q - mean
        ghat = gc_ / np.sqrt((gc_ * gc_).sum(-1, keepdims=True) + EPS)

        rs = np.concatenate(
            [(ghat @ Wk64).T, (KAPPA * KAPPA) * (Wk64.T @ mean)[:, None]],
            axis=1,
        )                                                   # [640, 6]
        rs8 = (32.0 * rs).astype(np.float32).astype(E4M3)
        r2 = (1024.0 * (rs - rs8.astype(np.float64) / 32.0)).astype(
            np.float32
        ).astype(E4M3)
        R = np.zeros((768, ZP), dtype=E4M3)
        R[:C, 0:ZN] = (KAPPA * Wk64.T).astype(np.float32).astype(E4M3)
        R[:C, ZN : ZN + 6] = rs8
        R[:C, ZN + 6 : ZN + 12] = r2
        rmat = np.ascontiguousarray(R.reshape(6, 128, ZP))

        consts = np.empty(NG + 1, np.float32)
        consts[0:NG] = 32.0 * (ghat @ mean)
        consts[NG] = 1024.0 * (ALPHA * ALPHA) * (mean @ mean + EPS)

        in_maps.append(
            {
                "xa": _block_x(a8),
                "xb": _block_x(b8),
                "rmat": rmat,
                "consts": consts,
                "bmat": bmat,
                "btmat": btmat,
            }
        )
    return in_maps


def kernel(global_f, local_f, Wq, Wk):
    in_maps = _host_prep(global_f, local_f, Wq, Wk)

    if not _PROGRAM_CACHE:
        _PROGRAM_CACHE.append(_build_program())
    nc = _PROGRAM_CACHE[0]

    res = run_bass_kernel_spmd(nc, in_maps, core_ids=list(range(B)))
    LAST_RESULTS.clear()
    LAST_RESULTS.append(res)

    out = np.empty((B, NL, NG, NF, 1), np.float32)
    for bi in range(B):
        Ob = res.results[bi]["O"][:, :NT]                   # [5, 14700]
        out[bi] = Ob.reshape(NG, NL, NF).transpose(1, 0, 2)[..., None]
    return out


# revision 56
# speedup vs baseline: 1.1395x; 1.1395x over previous
"""AttentiveMMDPrompt.compute_attn_weight kernel for 8 Trainium2 NeuronCores.

Strategy (data-parallel over episodes b=8, one episode per core):

  Per episode the heavy compute is the local projection l = local_f @ Wk^T
  (14700x640 @ 640x640).  Everything downstream needs only per-token
  scalars: |l|^2, the five raw scores l.ghat_g, and l.mean — all columns of
  one widened matmul Z = x @ R.

  The widened matmul runs in fp8 (e4m3) with DoubleRow perf mode: 0.5
  cycles/row, 2x the bf16/f32r rate.  fp8 alone is accurate enough for
  |l|^2 (quantization errors average out over 640 terms) but NOT for the
  score columns, so scores use a 3-term fp8 split that reaches bf16-level
  accuracy at fp8 speed:

    x  = a + b/32           a = fp8(x),         b = fp8(32(x - a))
    rs = rs8/32 + r2/1024   rs8 = fp8(32 rs),   r2 = fp8(1024(rs - rs8/32))
    x@rs ~= [a@rs8 + (a@r2 + b@rs8)/32] / 32    (second-order terms dropped)

  PSUM Z layout per 128-token chunk (two banks; a 652-col single matmul
  would cross the 2KB bank boundary, which the ISA rejects):
    bank0  0:320    kappa*l (norm, lo)  - squared+accumulated on ACT
    bank1  512:832  kappa*l (norm, hi)  - squared via DVE bn_stats, with
                                          the mean/var fold-in batched at
                                          phase level
    bank1  832:844  A1 | A2+B           - A1/A2 ride the A matmuls free;
                                          5 single-subtile fp8 B matmuls
                                          accumulate into the A2 columns
                                          (same 1/32 downstream scale;
                                          singles beat DoubleRow pairs here
                                          because their tiny streams cannot
                                          hide weight loads, and single
                                          loads are half the size)
  DVE copies the 12 score cols out per chunk; ln/exp on ACT replace
  sqrt+reciprocal so Square/Ln/Exp/Copy share one ACT function table
  (each table swap stalls ACT ~1.3us).

  Softmax groups (196 tokens) complete a few chunks after their last
  token chunk, so the whole phase epilogue (softmax stats, exp), the
  group sums, 1/sum and the broadcast-normalize matmuls for phase p are
  emitted as small steps interleaved between the chunks of phase p+1 —
  no engine sees a burst and the serial tail shrinks from ~75 chunks of
  work to ~16.  Group-sum matmuls are flipped (stationary = ebuf chunk,
  moving = indicator chunk, both bf16) so the big weight load sits on
  the cheap side; broadcast-normalize multiplies run batched 8 chunks
  per DVE op (PSUM-access setup dominates small ops).

  Device inputs per core (everything pre-laid-out on the host):
    xa     [128, 115, 6, 128] fp8 - a = fp8(x), partition-major blocks,
                                    c zero-padded 640->768 so each
                                    5-chunk group DMA is one contiguous
                                    3.84KB run per partition
    xb     [128, 115, 6, 128] fp8 - b = fp8(32(x - a)), same blocking
    rmat   [6, 128, 652] fp8     - [8*Wk^T | rs8 | r2], subtile 5 zeros
    consts [6] f32               - [32*ghat_g.mean (5) | 1024*a^2(|mean|^2+eps)]
    bmat   [128, 115, 75] bf16   - group indicator chunks, partition-major
    btmat  [75, 14720] bf16      - its transpose
  Output per core:
    O [5, 14720] f32 - normalized attention, token-major per g; host slices
    the 20 pad tokens and reshapes to [75, 5, 196, 1].
"""

import numpy as np
import ml_dtypes

import bass_rust
import concourse.bass as bass
import concourse.mybir as mybir
import concourse.tile as tile
from concourse.bass_utils import run_bass_kernel_spmd
from concourse.masks import make_identity

# Problem shapes (hardcoded per contract).
B, NG, NL, NF, C = 8, 5, 75, 196, 640
ALPHA, EPS = 0.1, 1e-12
NT = NL * NF            # 14700 tokens per episode
TCH = 115               # token chunks of 128
NTP = TCH * 128         # 14720 (padded)
CCH = 5                 # contraction chunks of 128 (C = 640)
KAPPA = 8.0             # fp8 prescale of Wk^T (norm path)
ZN = 640                # norm columns of Z
ZP = 652                # z columns: 640 norm | 6 A1 | 6 A2 (+6 B separate)
ZA = 320                # bank0 norm cols; bank1 holds 320:652 (+6 B cols)
G = 5                   # token chunks per DMA group (115 = 23*5)
NSLOT = 3               # x-stream SBUF slots
DEFER = 3               # chunks into phase p+1 before phase-p reductions
STEPS = 4               # deferred-work steps interleaved per chunk
PHASE_ENDS = [25, 50, 75, 100, TCH]
F32 = mybir.dt.float32
BF16 = mybir.dt.bfloat16
FP8 = mybir.dt.float8e4
AF = mybir.ActivationFunctionType
ALU = mybir.AluOpType
DR = mybir.MatmulPerfMode.DoubleRow
E4M3 = ml_dtypes.float8_e4m3


def _group_bounds():
    """Per phase: groups fully summed and chunks fully normalizable."""
    L, T = [], []
    for E in PHASE_ENDS:
        l = (128 * E) // NF          # groups with last token < 128E
        L.append(min(l, NL))
        if E == TCH:
            T.append(TCH)            # pad tokens have all-zero indicators
        else:
            T.append((NF * min(l, NL) - 128) // 128 + 1)
    return L, T


def _split_multi_waits(nc: bass.Bass) -> None:
    """Rewrite the BIR so no instruction carries more than one sem wait.

    The walrus build in this container rejects instructions with more than
    one sync-wait command (CoreV3 setupSyncWait, all encodings).  Extra
    waits are hoisted onto no-op instructions inserted immediately before
    the owner on the same engine: waits execute in program order per
    engine sequencer, so satisfying them one instruction earlier on the
    same engine is semantically identical.
    """
    for f in nc.m.functions:
        for b in f.blocks:
            insts = list(b.instructions)
            out = []
            changed = False
            for inst in insts:
                si = inst.sync_info
                if si is not None and len(si.on_wait) > 1:
                    waits = list(si.on_wait)
                    for w in waits[:-1]:
                        nop = mybir.InstNoOp(
                            name=nc.get_next_instruction_name(), ins=[], outs=[]
                        )
                        nop.engine = inst.engine
                        nop.sync_info = bass_rust.SyncInfo(
                            on_wait=[w], on_update=[]
                        )
                        nc.register_instruction(nop)
                        out.append(nop)
                    inst.sync_info = bass_rust.SyncInfo(
                        on_wait=[waits[-1]], on_update=list(si.on_update)
                    )
                    changed = True
                out.append(inst)
            if changed:
                b.instructions = out


def _build_program() -> bass.Bass:
    nc = bass.Bass(
        "TRN2",
        target_bir_lowering=False,
        debug=False,
        enable_asserts=True,
        num_devices=B,
    )
    xa = nc.dram_tensor("xa", [128, TCH, 6, 128], FP8, kind="ExternalInput")
    xb = nc.dram_tensor("xb", [128, TCH, 6, 128], FP8, kind="ExternalInput")
    rmat = nc.dram_tensor("rmat", [6, 128, ZP], FP8, kind="ExternalInput")
    consts = nc.dram_tensor("consts", [NG + 1], F32, kind="ExternalInput")
    bmat = nc.dram_tensor("bmat", [128, TCH, NL], BF16, kind="ExternalInput")
    btmat = nc.dram_tensor("btmat", [NL, NTP], BF16, kind="ExternalInput")
    O = nc.dram_tensor("O", [NG, NTP], F32, kind="ExternalOutput")

    LB, TB = _group_bounds()

    with tile.TileContext(nc, num_cores=B) as tc:
        with (
            tc.tile_pool(name="singles", bufs=1) as singles,
            tc.tile_pool(name="zpsum", bufs=3, space="PSUM") as zpsum,
            tc.tile_pool(name="gpsum", bufs=1, space="PSUM") as gpsum,
            tc.tile_pool(name="tpsum", bufs=1, space="PSUM") as tpsum,
        ):
            # ---- one-time loads -------------------------------------------------
            # x streams: persistent slot buffers.  Subtile 5 (the DoubleRow
            # partner of contraction chunk 4) is zero-padded on the host so
            # each 5-chunk group DMA is fully contiguous on both sides.
            xas = singles.tile([128, NSLOT, G, 6, 128], FP8)
            xbs = singles.tile([128, NSLOT, G, 6, 128], FP8)
            rm = singles.tile([128, 6, ZP], FP8)
            rmr = rmat.rearrange("s p z -> p s z")

            # Startup order: tiny rm first, then the first group's chunks
            # PER-CHUNK (so chunk 0's matmuls start after ~1/5 of a group),
            # then the rest.  Everything else queues behind these.
            nc.sync.dma_start(out=rm[:, 0:2, :], in_=rmr[:, 0:2, :])
            for j0 in range(G):
                for src, dst in ((xa, xas), (xb, xbs)):
                    nc.sync.dma_start(
                        out=dst[:, 0, j0, :, :], in_=src[:, j0, :, :]
                    )
            nc.sync.dma_start(out=rm[:, 2:6, :], in_=rmr[:, 2:6, :])

            bsb = singles.tile([128, TCH, NL], BF16)
            btsb = singles.tile([NL, TCH, 128], BF16)

            cg = singles.tile([128, NG], F32)
            nc.sync.dma_start(out=cg, in_=consts[0:NG].partition_broadcast(128))
            m2e = singles.tile([128, 1], F32)
            nc.sync.dma_start(
                out=m2e, in_=consts[NG : NG + 1].partition_broadcast(128)
            )

            ident = singles.tile([128, 128], F32)
            make_identity(nc, ident)

            # ---- persistent per-token stats -------------------------------------
            ssa = singles.tile([128, TCH], F32)       # kappa^2 |l|^2 per token
            sq12 = singles.tile([128, 12, TCH], F32)  # raw A1 | A2+B score cols
            sclm = singles.tile([128, 6, TCH], F32)   # combined 32*(x@rs)
            sqa = singles.tile([128, ZA], F32)        # ACT square scratch
            bno = singles.tile([128, 2, 3, TCH], F32)  # DVE bn_stats out
            bnm = singles.tile([128, 2, TCH], F32)    # bn mini scratch
            tmp0 = singles.tile([128, TCH], F32)
            nrm = singles.tile([128, TCH], F32)
            inv = singles.tile([128, TCH], F32)
            sfin = singles.tile([128, NG, TCH], F32)
            ebuf = singles.tile([128, NG, TCH], BF16)
            abuf = singles.tile([128, NG, TCH], BF16)
            obuf = singles.tile([TCH, NG, 128], F32)
            gsum = singles.tile([NG, NL], F32)        # group sums (g-major)
            rgsT = singles.tile([NG, NL], F32)        # 1/gsum (g-major)
            rgs = singles.tile([NL, NG], BF16)        # 1/gsum, group-major

            nc.vector.memset(gsum, 0)
            nc.vector.memset(rgsT, 0)

            def emit_chunk(t):
                gi, j = divmod(t, G)
                slot = gi % NSLOT
                if j == 0 and gi > 0:
                    for src, dst in ((xa, xas), (xb, xbs)):
                        nc.sync.dma_start(
                            out=dst[:, slot, :, :, :],
                            in_=src[:, G * gi : G * (gi + 1), :, :],
                        )
                pz = zpsum.tile([128, 1024], F32, tag="pz")
                for k in range(3):
                    pr = slice(2 * k, 2 * k + 2)
                    nc.tensor.matmul(
                        pz[:, 0:ZA],
                        xas[:, slot, j, pr, :],
                        rm[:, pr, 0:ZA],
                        start=(k == 0),
                        stop=(k == 2),
                        perf_mode=DR,
                    )
                    nc.tensor.matmul(
                        pz[:, 512 : 512 + (ZP - ZA)],
                        xas[:, slot, j, pr, :],
                        rm[:, pr, ZA:ZP],
                        start=(k == 0),
                        stop=(k == 2),
                        perf_mode=DR,
                    )
                # b@rs8 accumulates straight into the A2 columns (the two
                # correction terms share the same downstream 1/32 scale):
                # bank1's start-matmul zeroed 838:844, so these just add.
                for k in range(CCH):
                    nc.tensor.matmul(
                        pz[:, 838:844],
                        xbs[:, slot, j, k, :],
                        rm[:, k, ZN : ZN + 6],
                        start=False,
                        stop=(k == CCH - 1),
                        skip_group_check=True,
                    )
                # |kappa l|^2: ACT squares bank0 (320 cols, accum), DVE
                # covers bank1 (320 cols) via bn_stats; the mean/var fold-in
                # happens once per phase.
                nc.scalar.activation(
                    sqa,
                    pz[:, 0:ZA],
                    AF.Square,
                    accum_out=ssa[:, t : t + 1],
                )
                nc.vector.bn_stats(bno[:, :, :, t], pz[:, 512 : 512 + ZA])
                # raw score columns A1 | A2+B (12 cols, one copy).
                nc.vector.tensor_copy(sq12[:, :, t], pz[:, 832:844])

            def deferred_work(p):
                """Epilogue + reduction work for phase p, yielded as small
                steps to interleave between phase p+1's chunks so no engine
                sees a burst: softmax stats, exp, group-sum matmuls, 1/sum
                for newly-final groups, then broadcast + normalize for
                chunks whose groups are all final."""
                S = PHASE_ENDS[p - 1] if p else 0
                E = PHASE_ENDS[p]
                sl = slice(S, E)
                # fold the DVE bn halves into ssa: ssa += sum_h cv_h + 160 mu_h^2
                nc.vector.tensor_mul(
                    bnm[:, :, sl], bno[:, :, 1, sl], bno[:, :, 1, sl]
                )
                yield
                nc.vector.scalar_tensor_tensor(
                    out=bnm[:, :, sl],
                    in0=bnm[:, :, sl],
                    scalar=float(ZA // 2),
                    in1=bno[:, :, 2, sl],
                    op0=ALU.mult,
                    op1=ALU.add,
                )
                yield
                nc.vector.tensor_add(bnm[:, 0, sl], bnm[:, 0, sl], bnm[:, 1, sl])
                yield
                nc.vector.tensor_add(ssa[:, sl], ssa[:, sl], bnm[:, 0, sl])
                yield
                # combined score: sclm = A1 + (A2 + B)/32  (= 32 * x@rs)
                nc.vector.scalar_tensor_tensor(
                    out=sclm[:, :, sl],
                    in0=sq12[:, 6:12, sl],
                    scalar=1.0 / 32.0,
                    in1=sq12[:, 0:6, sl],
                    op0=ALU.mult,
                    op1=ALU.add,
                )
                yield
                nc.vector.scalar_tensor_tensor(
                    out=nrm[:, sl],
                    in0=sclm[:, 5, sl],
                    scalar=-1.0 / 16.0,
                    in1=ssa[:, sl],
                    op0=ALU.mult,
                    op1=ALU.add,
                )
                yield
                # 1/(32 a |l-mean|) = exp(-ln(scale*nrm + bias)/2): ln and
                # exp share one ACT function table with square/copy, so the
                # engine never swaps tables (a 1.3us stall each time).
                nc.scalar.activation(
                    tmp0[:, sl],
                    nrm[:, sl],
                    AF.Ln,
                    bias=m2e[:, 0:1],
                    scale=float(1024.0 * ALPHA * ALPHA / (KAPPA * KAPPA)),
                )
                yield
                nc.scalar.activation(inv[:, sl], tmp0[:, sl], AF.Exp, scale=-0.5)
                yield
                for g in range(NG):
                    nc.vector.scalar_tensor_tensor(
                        out=sfin[:, g, sl],
                        in0=sclm[:, g, sl],
                        scalar=cg[:, g : g + 1],
                        in1=inv[:, sl],
                        op0=ALU.subtract,
                        op1=ALU.mult,
                    )
                    yield
                nc.scalar.activation(ebuf[:, :, sl], sfin[:, :, sl], AF.Exp)
                yield
                # group sums for this phase's chunks
                gst = gpsum.tile([NG, NL], F32, tag="gs")
                for t in range(S, E):
                    nc.tensor.matmul(
                        gst[:, :],
                        ebuf[:, :, t],
                        bsb[:, t, :],
                        start=(t == S),
                        stop=(t == E - 1),
                    )
                    if t > S:
                        yield
                # fold into running sums; reciprocal of newly-final rows
                nc.vector.tensor_add(gsum, gsum, gst[:, :])
                l0 = LB[p - 1] if p else 0
                l1 = LB[p]
                nc.vector.reciprocal(rgsT[:, l0:l1], gsum[:, l0:l1])
                yield
                tpr = tpsum.tile([NL, NG], F32, tag="tail")
                nc.tensor.transpose(tpr[:, :], rgsT[:, :], ident[:NG, :NG])
                nc.vector.tensor_copy(rgs, tpr[:, :])
                yield
                # broadcast 1/sum to tokens + normalize, batched 8 chunks
                # per DVE multiply (PSUM-access setup dominates small ops)
                t0 = TB[p - 1] if p else 0
                ts = list(range(t0, TB[p]))
                for i in range(0, len(ts), 8):
                    bt = ts[i : i + 8]
                    r2 = tpsum.tile([128, 8, NG], F32, tag="tail")
                    for k2, t in enumerate(bt):
                        nc.tensor.matmul(
                            r2[:, k2, :], btsb[:, t, :], rgs[:, :],
                            start=True, stop=True,
                        )
                        yield
                    nc.vector.tensor_mul(
                        abuf[:, :, bt[0] : bt[0] + len(bt)],
                        ebuf[:, :, bt[0] : bt[0] + len(bt)],
                        r2[:, 0 : len(bt), :].rearrange("p t g -> p g t"),
                    )
                    yield

            # ---- main pass ------------------------------------------------------
            pendings = []
            ph_start = 0
            for p, ph_end in enumerate(PHASE_ENDS):
                for t in range(ph_start, ph_end):
                    emit_chunk(t)
                    # Drain deferred reduction work, a few steps per chunk.
                    # The queue persists across phase boundaries so a short
                    # phase never forces a serial burst of leftover steps.
                    if t >= ph_start + DEFER:
                        budget = STEPS
                        while pendings and budget > 0:
                            if next(pendings[0], StopIteration) is StopIteration:
                                pendings.pop(0)
                            else:
                                budget -= 1
                    if t == ph_start + 1:
                        # indicator slices for THIS phase's deferred work,
                        # issued piecewise so nothing waits on one big blob
                        # and startup x-groups keep queue priority.
                        nc.sync.dma_start(
                            out=bsb[:, ph_start:ph_end, :],
                            in_=bmat[:, ph_start:ph_end, :],
                        )
                        bt0 = TB[p - 1] if p else 0
                        nc.sync.dma_start(
                            out=btsb[:, bt0 : TB[p], :],
                            in_=btmat.rearrange("l (t p) -> l t p", p=128)[
                                :, bt0 : TB[p], :
                            ],
                        )
                pendings.append(deferred_work(p))
                ph_start = ph_end
            for gen in pendings:
                for _ in gen:
                    pass

            # ---- transpose to token-major and store -----------------------------
            # bf16 transposes run at 1 cyc/row, half the f32 rate.
            identb = singles.tile([128, 128], BF16)
            nc.vector.tensor_copy(identb, ident)
            for g in range(NG):
                tp = tpsum.tile([TCH, 128], BF16, tag="tail")
                nc.tensor.transpose(tp[:, :], abuf[:, g, :], identb[:, :])
                nc.scalar.copy(obuf[:, g, :], tp[:, :])
            nc.sync.dma_start(
                out=O.rearrange("g (t p) -> t g p", p=128), in_=obuf
            )

    _split_multi_waits(nc)
    return nc


_PROGRAM_CACHE: list = []
LAST_RESULTS: list = []


def _block_x(t8: np.ndarray) -> np.ndarray:
    """[NT, C] fp8 -> [128p, TCH, 6s, 128i] partition-major blocks.

    The c dim is zero-padded 640 -> 768 so subtile 5 (the DoubleRow
    partner of contraction chunk 4) streams as real zeros and each
    5-chunk group is one fully contiguous 3840B run per partition."""
    buf = np.zeros((NTP, 768), dtype=E4M3)
    buf[:NT, :C] = t8
    return np.ascontiguousarray(
        buf.reshape(TCH, 128, 6, 128).transpose(3, 0, 2, 1)
    )


def _host_prep(global_f, local_f, Wq, Wk):
    """Per-episode host-side constant prep + layout marshaling -> in_maps."""
    gf = np.asarray(global_f, dtype=np.float32)
    lf = np.asarray(local_f, dtype=np.float32)
    Wq64 = np.asarray(Wq, dtype=np.float64)
    Wk64 = np.asarray(Wk, dtype=np.float64)

    # Episode-independent device tensors.
    tok = np.arange(NTP)
    grp = tok // NF
    bmat_full = ((grp[:, None] == np.arange(NL)[None, :]) & (tok[:, None] < NT))
    bmat_full = bmat_full.astype(ml_dtypes.bfloat16)        # [14720, 75]
    bmat = np.ascontiguousarray(
        bmat_full.reshape(TCH, 128, NL).transpose(1, 0, 2)
    )                                                       # [128, 115, 75]
    btmat = np.ascontiguousarray(bmat_full.T)               # [75, 14720]

    in_maps = []
    for bi in range(B):
        x = lf[bi].reshape(NT, C)
        a8 = x.astype(E4M3)
        b8 = (32.0 * (x - a8.astype(np.float32))).astype(E4M3)

        x64 = x.astype(np.float64)
        q = gf[bi].astype(np.float64) @ Wq64.T              # [5, 640]
        mean = (q.sum(0) + x64.sum(0) @ Wk64.T) / (NG + NT)
        gc_ = q - mean
        ghat = gc_ / np.sqrt((gc_ * gc_).sum(-1, keepdims=True) + EPS)

        rs = np.concatenate(
            [(ghat @ Wk64).T, (KAPPA * KAPPA) * (Wk64.T @ mean)[:, None]],
            axis=1,
        )                                                   # [640, 6]
        rs8 = (32.0 * rs).astype(np.float32).astype(E4M3)
        r2 = (1024.0 * (rs - rs8.astype(np.float64) / 32.0)).astype(
            np.float32
        ).astype(E4M3)
        R = np.zeros((768, ZP), dtype=E4M3)
        R[:C, 0:ZN] = (KAPPA * Wk64.T).astype(np.float32).astype(E4M3)
        R[:C, ZN : ZN + 6] = rs8
        R[:C, ZN + 6 : ZN + 12] = r2
        rmat = np.ascontiguousarray(R.reshape(6, 128, ZP))

        consts = np.empty(NG + 1, np.float32)
        consts[0:NG] = 32.0 * (ghat @ mean)
        consts[NG] = 1024.0 * (ALPHA * ALPHA) * (mean @ mean + EPS)

        in_maps.append(
            {
                "xa": _block_x(a8),
                "xb": _block_x(b8),
                "rmat": rmat,
                "consts": consts,
                "bmat": bmat,
                "btmat": btmat,
            }
        )
    return in_maps


def kernel(global_f, local_f, Wq, Wk):
    in_maps = _host_prep(global_f, local_f, Wq, Wk)

    if not _PROGRAM_CACHE:
        _PROGRAM_CACHE.append(_build_program())
    nc = _PROGRAM_CACHE[0]

    res = run_bass_kernel_spmd(nc, in_maps, core_ids=list(range(B)))
    LAST_RESULTS.clear()
    LAST_RESULTS.append(res)

    out = np.empty((B, NL, NG, NF, 1), np.float32)
    for bi in range(B):
        Ob = res.results[bi]["O"][:, :NT]                   # [5, 14700]
        out[bi] = Ob.reshape(NG, NL, NF).transpose(1, 0, 2)[..., None]
    return out


# revision 57
# speedup vs baseline: 1.1500x; 1.0092x over previous
"""AttentiveMMDPrompt.compute_attn_weight kernel for 8 Trainium2 NeuronCores.

Strategy (data-parallel over episodes b=8, one episode per core):

  Per episode the heavy compute is the local projection l = local_f @ Wk^T
  (14700x640 @ 640x640).  Everything downstream needs only per-token
  scalars: |l|^2, the five raw scores l.ghat_g, and l.mean — all columns of
  one widened matmul Z = x @ R.

  The widened matmul runs in fp8 (e4m3) with DoubleRow perf mode: 0.5
  cycles/row, 2x the bf16/f32r rate.  fp8 alone is accurate enough for
  |l|^2 (quantization errors average out over 640 terms) but NOT for the
  score columns, so scores use a 3-term fp8 split that reaches bf16-level
  accuracy at fp8 speed:

    x  = a + b/32           a = fp8(x),         b = fp8(32(x - a))
    rs = rs8/32 + r2/1024   rs8 = fp8(32 rs),   r2 = fp8(1024(rs - rs8/32))
    x@rs ~= [a@rs8 + (a@r2 + b@rs8)/32] / 32    (second-order terms dropped)

  PSUM Z layout per 128-token chunk (two banks; a 652-col single matmul
  would cross the 2KB bank boundary, which the ISA rejects):
    bank0  0:320    kappa*l (norm, lo)  - squared+accumulated on ACT
    bank1  512:832  kappa*l (norm, hi)  - squared via DVE bn_stats, with
                                          the mean/var fold-in batched at
                                          phase level
    bank1  832:844  A1 | A2+B           - A1/A2 ride the A matmuls free;
                                          5 single-subtile fp8 B matmuls
                                          accumulate into the A2 columns
                                          (same 1/32 downstream scale;
                                          singles beat DoubleRow pairs here
                                          because their tiny streams cannot
                                          hide weight loads, and single
                                          loads are half the size)
  DVE copies the 12 score cols out per chunk; ln/exp on ACT replace
  sqrt+reciprocal so Square/Ln/Exp/Copy share one ACT function table
  (each table swap stalls ACT ~1.3us).

  Softmax groups (196 tokens) complete a few chunks after their last
  token chunk, so the whole phase epilogue (softmax stats, exp), the
  group sums, 1/sum and the broadcast-normalize matmuls for phase p are
  emitted as small steps interleaved between the chunks of phase p+1 —
  no engine sees a burst and the serial tail shrinks from ~75 chunks of
  work to ~16.  Group-sum matmuls are flipped (stationary = ebuf chunk,
  moving = indicator chunk, both bf16) so the big weight load sits on
  the cheap side; broadcast-normalize multiplies run batched 8 chunks
  per DVE op (PSUM-access setup dominates small ops).

  Device inputs per core (everything pre-laid-out on the host):
    xa     [128, 115, 6, 128] fp8 - a = fp8(x), partition-major blocks,
                                    c zero-padded 640->768 so each
                                    5-chunk group DMA is one contiguous
                                    3.84KB run per partition
    xb     [128, 115, 6, 128] fp8 - b = fp8(32(x - a)), same blocking
    rmat   [6, 128, 652] fp8     - [8*Wk^T | rs8 | r2], subtile 5 zeros
    consts [6] f32               - [32*ghat_g.mean (5) | 1024*a^2(|mean|^2+eps)]
    bmat   [128, 115, 75] bf16   - group indicator chunks, partition-major
    btmat  [75, 14720] bf16      - its transpose
  Output per core:
    O [5, 14720] f32 - normalized attention, token-major per g; host slices
    the 20 pad tokens and reshapes to [75, 5, 196, 1].
"""

import numpy as np
import ml_dtypes

import bass_rust
import concourse.bass as bass
import concourse.mybir as mybir
import concourse.tile as tile
from concourse.bass_utils import run_bass_kernel_spmd
from concourse.masks import make_identity

# Problem shapes (hardcoded per contract).
B, NG, NL, NF, C = 8, 5, 75, 196, 640
ALPHA, EPS = 0.1, 1e-12
NT = NL * NF            # 14700 tokens per episode
TCH = 115               # token chunks of 128
NTP = TCH * 128         # 14720 (padded)
CCH = 5                 # contraction chunks of 128 (C = 640)
KAPPA = 8.0             # fp8 prescale of Wk^T (norm path)
ZN = 640                # norm columns of Z
ZP = 652                # z columns: 640 norm | 6 A1 | 6 A2 (+6 B separate)
ZA = 320                # bank0 norm cols; bank1 holds 320:652 (+6 B cols)
G = 5                   # token chunks per DMA group (115 = 23*5)
NSLOT = 3               # x-stream SBUF slots
DEFER = 3               # chunks into phase p+1 before phase-p reductions
STEPS = 4               # deferred-work steps interleaved per chunk
PHASE_ENDS = [25, 50, 75, 100, TCH]
F32 = mybir.dt.float32
BF16 = mybir.dt.bfloat16
FP8 = mybir.dt.float8e4
AF = mybir.ActivationFunctionType
ALU = mybir.AluOpType
DR = mybir.MatmulPerfMode.DoubleRow
E4M3 = ml_dtypes.float8_e4m3


def _group_bounds():
    """Per phase: groups fully summed and chunks fully normalizable."""
    L, T = [], []
    for E in PHASE_ENDS:
        l = (128 * E) // NF          # groups with last token < 128E
        L.append(min(l, NL))
        if E == TCH:
            T.append(TCH)            # pad tokens have all-zero indicators
        else:
            T.append((NF * min(l, NL) - 128) // 128 + 1)
    return L, T


def _split_multi_waits(nc: bass.Bass) -> None:
    """Rewrite the BIR so no instruction carries more than one sem wait.

    The walrus build in this container rejects instructions with more than
    one sync-wait command (CoreV3 setupSyncWait, all encodings).  Extra
    waits are hoisted onto no-op instructions inserted immediately before
    the owner on the same engine: waits execute in program order per
    engine sequencer, so satisfying them one instruction earlier on the
    same engine is semantically identical.
    """
    for f in nc.m.functions:
        for b in f.blocks:
            insts = list(b.instructions)
            out = []
            changed = False
            for inst in insts:
                si = inst.sync_info
                if si is not None and len(si.on_wait) > 1:
                    waits = list(si.on_wait)
                    for w in waits[:-1]:
                        nop = mybir.InstNoOp(
                            name=nc.get_next_instruction_name(), ins=[], outs=[]
                        )
                        nop.engine = inst.engine
                        nop.sync_info = bass_rust.SyncInfo(
                            on_wait=[w], on_update=[]
                        )
                        nc.register_instruction(nop)
                        out.append(nop)
                    inst.sync_info = bass_rust.SyncInfo(
                        on_wait=[waits[-1]], on_update=list(si.on_update)
                    )
                    changed = True
                out.append(inst)
            if changed:
                b.instructions = out


def _build_program() -> bass.Bass:
    nc = bass.Bass(
        "TRN2",
        target_bir_lowering=False,
        debug=False,
        enable_asserts=True,
        num_devices=B,
    )
    xa = nc.dram_tensor("xa", [128, TCH, 6, 128], FP8, kind="ExternalInput")
    xb = nc.dram_tensor("xb", [128, TCH, 6, 128], FP8, kind="ExternalInput")
    rmat = nc.dram_tensor("rmat", [6, 128, ZP], FP8, kind="ExternalInput")
    consts = nc.dram_tensor("consts", [NG + 1], F32, kind="ExternalInput")
    bmat = nc.dram_tensor("bmat", [128, TCH, NL], BF16, kind="ExternalInput")
    btmat = nc.dram_tensor("btmat", [NL, NTP], BF16, kind="ExternalInput")
    O = nc.dram_tensor("O", [NG, NTP], F32, kind="ExternalOutput")

    LB, TB = _group_bounds()

    with tile.TileContext(nc, num_cores=B) as tc:
        with (
            tc.tile_pool(name="singles", bufs=1) as singles,
            tc.tile_pool(name="zpsum", bufs=3, space="PSUM") as zpsum,
            tc.tile_pool(name="gpsum", bufs=1, space="PSUM") as gpsum,
            tc.tile_pool(name="tpsum", bufs=1, space="PSUM") as tpsum,
        ):
            # ---- one-time loads -------------------------------------------------
            # x streams: persistent slot buffers.  Subtile 5 (the DoubleRow
            # partner of contraction chunk 4) is zero-padded on the host so
            # each 5-chunk group DMA is fully contiguous on both sides.
            xas = singles.tile([128, NSLOT, G, 6, 128], FP8)
            xbs = singles.tile([128, NSLOT, G, 6, 128], FP8)
            rm = singles.tile([128, 6, ZP], FP8)
            rmr = rmat.rearrange("s p z -> p s z")

            # Startup order matters doubly: each dma_start costs ~0.6us of
            # SERIAL descriptor-issue time on the Sync engine, so keep the
            # issue count low AND put everything chunk 0 needs (its x piece
            # and ALL of rm) ahead of bulkier transfers.
            nc.sync.dma_start(out=rm[:, 0:2, :], in_=rmr[:, 0:2, :])
            for src, dst in ((xa, xas), (xb, xbs)):
                nc.sync.dma_start(out=dst[:, 0, 0, :, :], in_=src[:, 0, :, :])
            nc.sync.dma_start(out=rm[:, 2:6, :], in_=rmr[:, 2:6, :])
            for src, dst in ((xa, xas), (xb, xbs)):
                nc.sync.dma_start(
                    out=dst[:, 0, 1:G, :, :], in_=src[:, 1:G, :, :]
                )

            bsb = singles.tile([128, TCH, NL], BF16)
            btsb = singles.tile([NL, TCH, 128], BF16)

            cg = singles.tile([128, NG], F32)
            nc.sync.dma_start(out=cg, in_=consts[0:NG].partition_broadcast(128))
            m2e = singles.tile([128, 1], F32)
            nc.sync.dma_start(
                out=m2e, in_=consts[NG : NG + 1].partition_broadcast(128)
            )

            ident = singles.tile([128, 128], F32)
            make_identity(nc, ident)

            # ---- persistent per-token stats -------------------------------------
            ssa = singles.tile([128, TCH], F32)       # kappa^2 |l|^2 per token
            sq12 = singles.tile([128, 12, TCH], F32)  # raw A1 | A2+B score cols
            sclm = singles.tile([128, 6, TCH], F32)   # combined 32*(x@rs)
            sqa = singles.tile([128, ZA], F32)        # ACT square scratch
            bno = singles.tile([128, 2, 3, TCH], F32)  # DVE bn_stats out
            bnm = singles.tile([128, 2, TCH], F32)    # bn mini scratch
            tmp0 = singles.tile([128, TCH], F32)
            nrm = singles.tile([128, TCH], F32)
            inv = singles.tile([128, TCH], F32)
            sfin = singles.tile([128, NG, TCH], F32)
            ebuf = singles.tile([128, NG, TCH], BF16)
            abuf = singles.tile([128, NG, TCH], BF16)
            obuf = singles.tile([TCH, NG, 128], F32)
            gsum = singles.tile([NG, NL], F32)        # group sums (g-major)
            rgsT = singles.tile([NG, NL], F32)        # 1/gsum (g-major)
            rgs = singles.tile([NL, NG], BF16)        # 1/gsum, group-major

            nc.vector.memset(gsum, 0)
            nc.vector.memset(rgsT, 0)

            def emit_chunk(t):
                gi, j = divmod(t, G)
                slot = gi % NSLOT
                if j == 0 and gi > 0:
                    for src, dst in ((xa, xas), (xb, xbs)):
                        nc.sync.dma_start(
                            out=dst[:, slot, :, :, :],
                            in_=src[:, G * gi : G * (gi + 1), :, :],
                        )
                pz = zpsum.tile([128, 1024], F32, tag="pz")
                for k in range(3):
                    pr = slice(2 * k, 2 * k + 2)
                    nc.tensor.matmul(
                        pz[:, 0:ZA],
                        xas[:, slot, j, pr, :],
                        rm[:, pr, 0:ZA],
                        start=(k == 0),
                        stop=(k == 2),
                        perf_mode=DR,
                    )
                    nc.tensor.matmul(
                        pz[:, 512 : 512 + (ZP - ZA)],
                        xas[:, slot, j, pr, :],
                        rm[:, pr, ZA:ZP],
                        start=(k == 0),
                        stop=(k == 2),
                        perf_mode=DR,
                    )
                # b@rs8 accumulates straight into the A2 columns (the two
                # correction terms share the same downstream 1/32 scale):
                # bank1's start-matmul zeroed 838:844, so these just add.
                for k in range(CCH):
                    nc.tensor.matmul(
                        pz[:, 838:844],
                        xbs[:, slot, j, k, :],
                        rm[:, k, ZN : ZN + 6],
                        start=False,
                        stop=(k == CCH - 1),
                        skip_group_check=True,
                    )
                # |kappa l|^2: ACT squares bank0 (320 cols, accum), DVE
                # covers bank1 (320 cols) via bn_stats; the mean/var fold-in
                # happens once per phase.
                nc.scalar.activation(
                    sqa,
                    pz[:, 0:ZA],
                    AF.Square,
                    accum_out=ssa[:, t : t + 1],
                )
                nc.vector.bn_stats(bno[:, :, :, t], pz[:, 512 : 512 + ZA])
                # raw score columns A1 | A2+B (12 cols, one copy).
                nc.vector.tensor_copy(sq12[:, :, t], pz[:, 832:844])

            def deferred_work(p):
                """Epilogue + reduction work for phase p, yielded as small
                steps to interleave between phase p+1's chunks so no engine
                sees a burst: softmax stats, exp, group-sum matmuls, 1/sum
                for newly-final groups, then broadcast + normalize for
                chunks whose groups are all final."""
                S = PHASE_ENDS[p - 1] if p else 0
                E = PHASE_ENDS[p]
                sl = slice(S, E)
                # fold the DVE bn halves into ssa: ssa += sum_h cv_h + 160 mu_h^2
                nc.vector.tensor_mul(
                    bnm[:, :, sl], bno[:, :, 1, sl], bno[:, :, 1, sl]
                )
                yield
                nc.vector.scalar_tensor_tensor(
                    out=bnm[:, :, sl],
                    in0=bnm[:, :, sl],
                    scalar=float(ZA // 2),
                    in1=bno[:, :, 2, sl],
                    op0=ALU.mult,
                    op1=ALU.add,
                )
                yield
                nc.vector.tensor_add(bnm[:, 0, sl], bnm[:, 0, sl], bnm[:, 1, sl])
                yield
                nc.vector.tensor_add(ssa[:, sl], ssa[:, sl], bnm[:, 0, sl])
                yield
                # combined score: sclm = A1 + (A2 + B)/32  (= 32 * x@rs)
                nc.vector.scalar_tensor_tensor(
                    out=sclm[:, :, sl],
                    in0=sq12[:, 6:12, sl],
                    scalar=1.0 / 32.0,
                    in1=sq12[:, 0:6, sl],
                    op0=ALU.mult,
                    op1=ALU.add,
                )
                yield
                nc.vector.scalar_tensor_tensor(
                    out=nrm[:, sl],
                    in0=sclm[:, 5, sl],
                    scalar=-1.0 / 16.0,
                    in1=ssa[:, sl],
                    op0=ALU.mult,
                    op1=ALU.add,
                )
                yield
                # 1/(32 a |l-mean|) = exp(-ln(scale*nrm + bias)/2): ln and
                # exp share one ACT function table with square/copy, so the
                # engine never swaps tables (a 1.3us stall each time).
                nc.scalar.activation(
                    tmp0[:, sl],
                    nrm[:, sl],
                    AF.Ln,
                    bias=m2e[:, 0:1],
                    scale=float(1024.0 * ALPHA * ALPHA / (KAPPA * KAPPA)),
                )
                yield
                nc.scalar.activation(inv[:, sl], tmp0[:, sl], AF.Exp, scale=-0.5)
                yield
                for g in range(NG):
                    nc.vector.scalar_tensor_tensor(
                        out=sfin[:, g, sl],
                        in0=sclm[:, g, sl],
                        scalar=cg[:, g : g + 1],
                        in1=inv[:, sl],
                        op0=ALU.subtract,
                        op1=ALU.mult,
                    )
                    yield
                nc.scalar.activation(ebuf[:, :, sl], sfin[:, :, sl], AF.Exp)
                yield
                # group sums for this phase's chunks
                gst = gpsum.tile([NG, NL], F32, tag="gs")
                for t in range(S, E):
                    nc.tensor.matmul(
                        gst[:, :],
                        ebuf[:, :, t],
                        bsb[:, t, :],
                        start=(t == S),
                        stop=(t == E - 1),
                    )
                    if t > S:
                        yield
                # fold into running sums; reciprocal of newly-final rows
                nc.vector.tensor_add(gsum, gsum, gst[:, :])
                l0 = LB[p - 1] if p else 0
                l1 = LB[p]
                nc.vector.reciprocal(rgsT[:, l0:l1], gsum[:, l0:l1])
                yield
                tpr = tpsum.tile([NL, NG], F32, tag="tail")
                nc.tensor.transpose(tpr[:, :], rgsT[:, :], ident[:NG, :NG])
                nc.vector.tensor_copy(rgs, tpr[:, :])
                yield
                # broadcast 1/sum to tokens + normalize, batched 8 chunks
                # per DVE multiply (PSUM-access setup dominates small ops)
                t0 = TB[p - 1] if p else 0
                ts = list(range(t0, TB[p]))
                for i in range(0, len(ts), 8):
                    bt = ts[i : i + 8]
                    r2 = tpsum.tile([128, 8, NG], F32, tag="tail")
                    for k2, t in enumerate(bt):
                        nc.tensor.matmul(
                            r2[:, k2, :], btsb[:, t, :], rgs[:, :],
                            start=True, stop=True,
                        )
                        yield
                    nc.vector.tensor_mul(
                        abuf[:, :, bt[0] : bt[0] + len(bt)],
                        ebuf[:, :, bt[0] : bt[0] + len(bt)],
                        r2[:, 0 : len(bt), :].rearrange("p t g -> p g t"),
                    )
                    yield

            # ---- main pass ------------------------------------------------------
            pendings = []
            ph_start = 0
            for p, ph_end in enumerate(PHASE_ENDS):
                for t in range(ph_start, ph_end):
                    emit_chunk(t)
                    # Drain deferred reduction work, a few steps per chunk.
                    # The queue persists across phase boundaries so a short
                    # phase never forces a serial burst of leftover steps.
                    if t >= ph_start + DEFER:
                        budget = STEPS
                        while pendings and budget > 0:
                            if next(pendings[0], StopIteration) is StopIteration:
                                pendings.pop(0)
                            else:
                                budget -= 1
                    if t == ph_start + 1:
                        # indicator slices for THIS phase's deferred work,
                        # issued piecewise so nothing waits on one big blob
                        # and startup x-groups keep queue priority.
                        nc.sync.dma_start(
                            out=bsb[:, ph_start:ph_end, :],
                            in_=bmat[:, ph_start:ph_end, :],
                        )
                        bt0 = TB[p - 1] if p else 0
                        nc.sync.dma_start(
                            out=btsb[:, bt0 : TB[p], :],
                            in_=btmat.rearrange("l (t p) -> l t p", p=128)[
                                :, bt0 : TB[p], :
                            ],
                        )
                pendings.append(deferred_work(p))
                ph_start = ph_end
            for gen in pendings:
                for _ in gen:
                    pass

            # ---- transpose to token-major and store -----------------------------
            # bf16 transposes run at 1 cyc/row, half the f32 rate.
            identb = singles.tile([128, 128], BF16)
            nc.vector.tensor_copy(identb, ident)
            for g in range(NG):
                tp = tpsum.tile([TCH, 128], BF16, tag="tail")
                nc.tensor.transpose(tp[:, :], abuf[:, g, :], identb[:, :])
                nc.scalar.copy(obuf[:, g, :], tp[:, :])
            nc.sync.dma_start(
                out=O.rearrange("g (t p) -> t g p", p=128), in_=obuf
            )

    _split_multi_waits(nc)
    return nc


_PROGRAM_CACHE: list = []
LAST_RESULTS: list = []


def _block_x(t8: np.ndarray) -> np.ndarray:
    """[NT, C] fp8 -> [128p, TCH, 6s, 128i] partition-major blocks.

    The c dim is zero-padded 640 -> 768 so subtile 5 (the DoubleRow
    partner of contraction chunk 4) streams as real zeros and each
    5-chunk group is one fully contiguous 3840B run per partition."""
    buf = np.zeros((NTP, 768), dtype=E4M3)
    buf[:NT, :C] = t8
    return np.ascontiguousarray(
        buf.reshape(TCH, 128, 6, 128).transpose(3, 0, 2, 1)
    )


def _host_prep(global_f, local_f, Wq, Wk):
    """Per-episode host-side constant prep + layout marshaling -> in_maps."""
    gf = np.asarray(global_f, dtype=np.float32)
    lf = np.asarray(local_f, dtype=np.float32)
    Wq64 = np.asarray(Wq, dtype=np.float64)
    Wk64 = np.asarray(Wk, dtype=np.float64)

    # Episode-independent device tensors.
    tok = np.arange(NTP)
    grp = tok // NF
    bmat_full = ((grp[:, None] == np.arange(NL)[None, :]) & (tok[:, None] < NT))
    bmat_full = bmat_full.astype(ml_dtypes.bfloat16)        # [14720, 75]
    bmat = np.ascontiguousarray(
        bmat_full.reshape(TCH, 128, NL).transpose(1, 0, 2)
    )                                                       # [128, 115, 75]
    btmat = np.ascontiguousarray(bmat_full.T)               # [75, 14720]

    in_maps = []
    for bi in range(B):
        x = lf[bi].reshape(NT, C)
        a8 = x.astype(E4M3)
        b8 = (32.0 * (x - a8.astype(np.float32))).astype(E4M3)

        x64 = x.astype(np.float64)
        q = gf[bi].astype(np.float64) @ Wq64.T              # [5, 640]
        mean = (q.sum(0) + x64.sum(0) @ Wk64.T) / (NG + NT)
        gc_ = q - mean
        ghat = gc_ / np.sqrt((gc_ * gc_).sum(-1, keepdims=True) + EPS)

        rs = np.concatenate(
            [(ghat @ Wk64).T, (KAPPA * KAPPA) * (Wk64.T @ mean)[:, None]],
            axis=1,
        )                                                   # [640, 6]
        rs8 = (32.0 * rs).astype(np.float32).astype(E4M3)
        r2 = (1024.0 * (rs - rs8.astype(np.float64) / 32.0)).astype(
            np.float32
        ).astype(E4M3)
        R = np.zeros((768, ZP), dtype=E4M3)
        R[:C, 0:ZN] = (KAPPA * Wk64.T).astype(np.float32).astype(E4M3)
        R[:C, ZN : ZN + 6] = rs8
        R[:C, ZN + 6 : ZN + 12] = r2
        rmat = np.ascontiguousarray(R.reshape(6, 128, ZP))

        consts = np.empty(NG + 1, np.float32)
        consts[0:NG] = 32.0 * (ghat @ mean)
        consts[NG] = 1024.0 * (ALPHA * ALPHA) * (mean @ mean + EPS)

        in_maps.append(
            {
                "xa": _block_x(a8),
                "xb": _block_x(b8),
                "rmat": rmat,
                "consts": consts,
                "bmat": bmat,
                "btmat": btmat,
            }
        )
    return in_maps


def kernel(global_f, local_f, Wq, Wk):
    in_maps = _host_prep(global_f, local_f, Wq, Wk)

    if not _PROGRAM_CACHE:
        _PROGRAM_CACHE.append(_build_program())
    nc = _PROGRAM_CACHE[0]

    res = run_bass_kernel_spmd(nc, in_maps, core_ids=list(range(B)))
    LAST_RESULTS.clear()
    LAST_RESULTS.append(res)

    out = np.empty((B, NL, NG, NF, 1), np.float32)
    for bi in range(B):
        Ob = res.results[bi]["O"][:, :NT]                   # [5, 14700]
        out[bi] = Ob.reshape(NG, NL, NF).transpose(1, 0, 2)[..., None]
    return out
